# revision 3
# baseline (speedup 1.0000x reference)
"""Trainium2 Bass kernel for a dense transformer block (causal MHA + FFN).

Sharding: tensor-parallel over the 8 attention heads (1 head per core) for
the attention part, data-parallel over tokens for the FFN part, glued by a
ReduceScatter(add) of the per-head output-projection partials.

Shapes (hardcoded): B=2, T=4096, D=512, H=8, HS=64, FF=2048. 8 NeuronCores.
"""

import numpy as np
import ml_dtypes

import concourse.bass as bass
import concourse.mybir as mybir
from concourse import bacc
from concourse.tile import TileContext
from concourse import bass_utils
from concourse.masks import make_identity

B, T, D, H, HS = 2, 4096, 512, 8, 64
FF = 4 * D
NC = 8
TALL = B * T          # 8192 global tokens, batch-major
NTB = TALL // 512     # 16 token blocks of 512
NTT = TALL // 128     # 64 token tiles of 128
QB = T // 512         # 8 q-blocks of 512 per batch
KT = T // 128         # 32 k-tiles of 128 per batch
NG = 4                # ReduceScatter groups (2048 tokens each)
OWN = TALL // NC      # 1024 tokens owned per core post-RS

BF16 = mybir.dt.bfloat16
F32 = mybir.dt.float32
F16 = mybir.dt.float16

_CACHE = {}


def build_nc(iters: int = 1):
    nc = bacc.Bacc("TRN2", target_bir_lowering=False, debug=False, num_devices=NC)

    xT = nc.dram_tensor("xT", [D, TALL], BF16, kind="ExternalInput")
    wq = nc.dram_tensor("wq", [128, 4 * HS], BF16, kind="ExternalInput")
    wk = nc.dram_tensor("wk", [128, 4 * HS], BF16, kind="ExternalInput")
    wv = nc.dram_tensor("wv", [128, 4 * HS], BF16, kind="ExternalInput")
    wproj = nc.dram_tensor("wproj", [HS, D], BF16, kind="ExternalInput")
    w1 = nc.dram_tensor("w1", [128, 4 * FF], BF16, kind="ExternalInput")
    w2 = nc.dram_tensor("w2", [128, 16 * D], BF16, kind="ExternalInput")
    b1t = nc.dram_tensor("b1t", [128, 16], F32, kind="ExternalInput")
    b2t = nc.dram_tensor("b2t", [128, D], F32, kind="ExternalInput")
    xres = nc.dram_tensor("xres", [OWN, D], F32, kind="ExternalInput")
    maskm = nc.dram_tensor("maskm", [128, 4 * 512], BF16, kind="ExternalInput")
    out = nc.dram_tensor("out", [OWN, D], F32, kind="ExternalOutput")

    with TileContext(nc) as tc:
        with (
            tc.tile_pool(name="const", bufs=1) as constp,
            tc.tile_pool(name="qkv", bufs=1) as qkvp,
            tc.tile_pool(name="dram", bufs=1, space="DRAM") as dramp,
        ):
            # persistent constants
            wq_s = constp.tile([128, 4 * HS], BF16, tag="wq")
            wk_s = constp.tile([128, 4 * HS], BF16, tag="wk")
            wv_s = constp.tile([128, 4 * HS], BF16, tag="wv")
            wproj_s = constp.tile([HS, D], BF16, tag="wproj")
            maskm_s = constp.tile([128, 4 * 512], BF16, tag="maskm")
            ident = constp.tile([128, 128], BF16, tag="ident")
            nc.sync.dma_start(out=wq_s[:], in_=wq[:])
            nc.sync.dma_start(out=wk_s[:], in_=wk[:])
            nc.sync.dma_start(out=wv_s[:], in_=wv[:])
            nc.sync.dma_start(out=wproj_s[:], in_=wproj[:])
            nc.sync.dma_start(out=maskm_s[:], in_=maskm[:])
            make_identity(nc, ident[:])

            # persistent per-head attention tensors
            qT = qkvp.tile([64, TALL], BF16, tag="qT")     # Q^T  (d-major)
            kT = qkvp.tile([64, TALL], BF16, tag="kT")     # K^T  (d-major)
            vP = qkvp.tile([128, NTT * (HS + 1)], BF16, tag="vP")  # V tiles + ones col
            denT = qkvp.tile([128, NTT], F32, tag="denT")
            recT = qkvp.tile([128, NTT], F32, tag="recT")

            # RS dram buffers
            rs_in = [dramp.tile([2048, D], F16, tag=f"rsin{g}", name=f"rsin{g}") for g in range(NG)]
            rs_out = [dramp.tile([2048 // NC, D], F16, tag=f"rsout{g}", name=f"rsout{g}") for g in range(NG)]

            for _ in range(iters):
                # ---------------- Phase 1: projections ----------------
                with (
                    tc.tile_pool(name="xt", bufs=1) as xtp,
                    tc.tile_pool(name="p1q", bufs=2, space="PSUM") as p1q,
                    tc.tile_pool(name="p1v", bufs=2, space="PSUM") as p1v,
                ):
                    xt_s = [xtp.tile([128, TALL], BF16, tag=f"xt{di}", name=f"xt{di}") for di in range(4)]
                    for di in range(4):
                        nc.sync.dma_start(
                            out=xt_s[di][:], in_=xT[128 * di : 128 * (di + 1), :]
                        )
                    # ones columns of V' (col HS of each 128-token tile)
                    nc.vector.memset(vP[:, HS :: HS + 1], 1.0)

                    for tb in range(NTB):
                        sl = slice(512 * tb, 512 * (tb + 1))
                        pq = p1q.tile([64, 512], F32, tag="pq")
                        for di in range(4):
                            nc.tensor.matmul(
                                pq[:],
                                wq_s[:, HS * di : HS * (di + 1)],
                                xt_s[di][:, sl],
                                start=(di == 0),
                                stop=(di == 3),
                            )
                        nc.vector.tensor_copy(qT[:, sl], pq[:])
                        pk = p1q.tile([64, 512], F32, tag="pq")
                        for di in range(4):
                            nc.tensor.matmul(
                                pk[:],
                                wk_s[:, HS * di : HS * (di + 1)],
                                xt_s[di][:, sl],
                                start=(di == 0),
                                stop=(di == 3),
                            )
                        nc.vector.tensor_copy(kT[:, sl], pk[:])
                        # V for the 4 token tiles of this block
                        for tt4 in range(4):
                            tt = 4 * tb + tt4
                            pv = p1v.tile([128, HS], F32, tag="pv")
                            for di in range(4):
                                nc.tensor.matmul(
                                    pv[:],
                                    xt_s[di][:, 128 * tt : 128 * (tt + 1)],
                                    wv_s[:, HS * di : HS * (di + 1)],
                                    start=(di == 0),
                                    stop=(di == 3),
                                )
                            nc.vector.tensor_copy(
                                vP[:, (HS + 1) * tt : (HS + 1) * tt + HS], pv[:]
                            )

                # ---------------- Phase 2..4 ----------------
                with (
                    tc.tile_pool(name="sps", bufs=2, space="PSUM") as sps,
                    tc.tile_pool(name="avps", bufs=2, space="PSUM") as avps,
                    tc.tile_pool(name="ppps", bufs=1, space="PSUM") as ppps,
                    tc.tile_pool(name="trps", bufs=1, space="PSUM") as trps,
                    tc.tile_pool(name="f1ps", bufs=1, space="PSUM") as f1ps,
                    tc.tile_pool(name="f2ps", bufs=1, space="PSUM") as f2ps,
                    tc.tile_pool(name="work", bufs=4) as workp,
                    tc.tile_pool(name="work2", bufs=2) as work2p,
                    tc.tile_pool(name="big2", bufs=1) as big2p,
                ):
                    xres_s = big2p.tile([128, 8 * D], F32, tag="xres")
                    nc.sync.dma_start(
                        out=xres_s[:],
                        in_=xres.ap().rearrange("(a p) d -> p a d", p=128),
                    )
                    w1_s = big2p.tile([128, 4 * FF], BF16, tag="w1")
                    w2_s = big2p.tile([128, 16 * D], BF16, tag="w2")
                    b1t_s = big2p.tile([128, 16], F32, tag="b1t")
                    b2t_s = big2p.tile([128, D], F32, tag="b2t")
                    nc.sync.dma_start(out=w1_s[:], in_=w1[:])
                    nc.sync.dma_start(out=w2_s[:], in_=w2[:])
                    nc.sync.dma_start(out=b1t_s[:], in_=b1t[:])
                    nc.sync.dma_start(out=b2t_s[:], in_=b2t[:])
                    xmid = big2p.tile([128, 8 * D], F32, tag="xmid")
                    xmidT = big2p.tile([128, 4 * OWN], BF16, tag="xmidT")
                    h1T = big2p.tile([128, 16 * OWN], BF16, tag="h1T")

                    def attention(b, qb):
                        qsl = slice(T * b + 512 * qb, T * b + 512 * (qb + 1))
                        nkt = 4 * qb + 4
                        av = avps.tile([HS + 1, 512], F32, tag="av")
                        for kt in range(nkt):
                            gt = KT * b + kt  # global 128-token tile
                            s = sps.tile([128, 512], F32, tag="s")
                            nc.tensor.matmul(
                                s[:],
                                kT[:, 128 * gt : 128 * (gt + 1)],
                                qT[:, qsl],
                                start=True,
                                stop=True,
                            )
                            pT = workp.tile([128, 512], BF16, tag="pT")
                            nc.scalar.activation(
                                pT[:], s[:], mybir.ActivationFunctionType.Exp
                            )
                            if kt >= 4 * qb:
                                j = kt - 4 * qb
                                nc.vector.tensor_mul(
                                    pT[:], pT[:], maskm_s[:, 512 * j : 512 * (j + 1)]
                                )
                            nc.tensor.matmul(
                                av[:],
                                vP[:, (HS + 1) * gt : (HS + 1) * (gt + 1)],
                                pT[:],
                                start=(kt == 0),
                                stop=(kt == nkt - 1),
                            )
                        attnT = work2p.tile([64, 512], BF16, tag="attnT")
                        nc.vector.tensor_copy(attnT[:], av[0:HS, :])
                        drow = work2p.tile([1, 512], F32, tag="drow")
                        nc.vector.tensor_copy(drow[:], av[HS : HS + 1, :])
                        g = 2 * b + qb // 4
                        for tt in range(4):
                            gtt = 32 * b + 4 * qb + tt  # global token tile
                            nc.sync.dma_start(
                                out=denT[:, gtt : gtt + 1],
                                in_=drow[0:1, 128 * tt : 128 * (tt + 1)],
                            )
                        nc.vector.reciprocal(
                            recT[:, 32 * b + 4 * qb : 32 * b + 4 * qb + 4],
                            denT[:, 32 * b + 4 * qb : 32 * b + 4 * qb + 4],
                        )
                        for tt in range(4):
                            gtt = 32 * b + 4 * qb + tt
                            pp = ppps.tile([128, D], F32, tag="pp")
                            nc.tensor.matmul(
                                pp[:],
                                attnT[:, 128 * tt : 128 * (tt + 1)],
                                wproj_s[:],
                                start=True,
                                stop=True,
                            )
                            stg = workp.tile([128, D], F16, tag="stg")
                            nc.vector.tensor_scalar_mul(
                                stg[:], pp[:], recT[:, gtt : gtt + 1]
                            )
                            roff = (512 * qb + 128 * tt) % 2048
                            nc.sync.dma_start(
                                out=rs_in[g][roff : roff + 128, :], in_=stg[:]
                            )

                    def rs_fire(g):
                        nc.gpsimd.collective_compute(
                            "ReduceScatter",
                            mybir.AluOpType.add,
                            replica_groups=[list(range(NC))],
                            ins=[rs_in[g].opt()],
                            outs=[rs_out[g].opt()],
                        )

                    def post_rs(g):
                        # rs_out[g]: [256, D] f16 -> sbuf [128, 2*D]
                        ro = work2p.tile([128, 2 * D], F16, tag="ro")
                        nc.sync.dma_start(
                            out=ro[:],
                            in_=rs_out[g].rearrange("(a p) d -> p a d", p=128),
                        )
                        for t2 in range(2):
                            ti = 2 * g + t2  # owned token tile 0..7
                            csl = slice(D * ti, D * (ti + 1))
                            nc.vector.tensor_add(
                                xmid[:, csl],
                                ro[:, D * t2 : D * (t2 + 1)],
                                xres_s[:, csl],
                            )
                            xb = workp.tile([128, D], BF16, tag="xb")
                            nc.vector.tensor_copy(xb[:], xmid[:, csl])
                            for di in range(4):
                                tr = trps.tile([128, 128], BF16, tag="tr")
                                nc.tensor.transpose(
                                    tr[:], xb[:, 128 * di : 128 * (di + 1)], ident[:]
                                )
                                nc.vector.tensor_copy(
                                    xmidT[:, OWN * di + 128 * ti : OWN * di + 128 * (ti + 1)],
                                    tr[:],
                                )

                    def ffn(th):
                        tsl = slice(512 * th, 512 * (th + 1))
                        for hi in range(16):
                            f1 = f1ps.tile([128, 512], F32, tag="f1")
                            for di in range(4):
                                nc.tensor.matmul(
                                    f1[:],
                                    w1_s[:, FF * di + 128 * hi : FF * di + 128 * (hi + 1)],
                                    xmidT[:, OWN * di + 512 * th : OWN * di + 512 * (th + 1)],
                                    start=(di == 0),
                                    stop=(di == 3),
                                )
                            nc.vector.tensor_scalar(
                                h1T[:, OWN * hi + 512 * th : OWN * hi + 512 * (th + 1)],
                                f1[:],
                                b1t_s[:, hi : hi + 1],
                                0.0,
                                op0=mybir.AluOpType.add,
                                op1=mybir.AluOpType.max,
                            )
                        for tt in range(4):
                            ti = 4 * th + tt
                            f2 = f2ps.tile([128, D], F32, tag="f2")
                            for hi in range(16):
                                nc.tensor.matmul(
                                    f2[:],
                                    h1T[:, OWN * hi + 128 * ti : OWN * hi + 128 * (ti + 1)],
                                    w2_s[:, D * hi : D * (hi + 1)],
                                    start=(hi == 0),
                                    stop=(hi == 15),
                                )
                            osum = workp.tile([128, D], F32, tag="osum")
                            nc.vector.tensor_add(osum[:], f2[:], xmid[:, D * ti : D * (ti + 1)])
                            nc.vector.tensor_add(osum[:], osum[:], b2t_s[:])
                            nc.sync.dma_start(
                                out=out[128 * ti : 128 * (ti + 1), :], in_=osum[:]
                            )

                    for b in range(B):
                        for qb in range(QB):
                            attention(b, qb)
                            if qb % 4 == 3:
                                g = 2 * b + qb // 4
                                rs_fire(g)
                                post_rs(g)
                                if g % 2 == 1:
                                    ffn(g // 2)

    nc.compile()
    return nc


def _prep_inputs(x, Wq, Wk, Wv, Wproj, bproj, W1, b1, W2, b2):
    bf16 = ml_dtypes.bfloat16
    xf = np.ascontiguousarray(x.reshape(TALL, D).astype(np.float32))
    xT_np = np.ascontiguousarray(xf.T).astype(bf16)
    w1_np = np.ascontiguousarray(
        W1.reshape(4, 128, FF).transpose(1, 0, 2).reshape(128, 4 * FF)
    ).astype(bf16)
    w2_np = np.ascontiguousarray(
        W2.reshape(16, 128, D).transpose(1, 0, 2).reshape(128, 16 * D)
    ).astype(bf16)
    b1t_np = np.ascontiguousarray(b1.reshape(16, 128).T).astype(np.float32)
    b2t_np = np.ascontiguousarray(np.broadcast_to(b2, (128, D))).astype(np.float32)

    ki = np.arange(128)[:, None]
    qi = np.arange(512)[None, :]
    maskm_np = np.concatenate(
        [(qi >= 128 * j + ki) for j in range(4)], axis=1
    ).astype(bf16)

    scale = HS ** -0.5
    in_maps = []
    for c in range(NC):
        wq_np = np.ascontiguousarray(
            (Wq[c] * scale).reshape(4, 128, HS).transpose(1, 0, 2).reshape(128, 4 * HS)
        ).astype(bf16)
        wk_np = np.ascontiguousarray(
            Wk[c].reshape(4, 128, HS).transpose(1, 0, 2).reshape(128, 4 * HS)
        ).astype(bf16)
        wv_np = np.ascontiguousarray(
            Wv[c].reshape(4, 128, HS).transpose(1, 0, 2).reshape(128, 4 * HS)
        ).astype(bf16)
        wproj_np = np.ascontiguousarray(Wproj[HS * c : HS * (c + 1), :]).astype(bf16)
        xres_np = np.concatenate(
            [xf[2048 * g + 256 * c : 2048 * g + 256 * (c + 1)] for g in range(NG)]
        ) + bproj.astype(np.float32)
        in_maps.append(
            {
                "xT": xT_np,
                "wq": wq_np,
                "wk": wk_np,
                "wv": wv_np,
                "wproj": wproj_np,
                "w1": w1_np,
                "w2": w2_np,
                "b1t": b1t_np,
                "b2t": b2t_np,
                "xres": np.ascontiguousarray(xres_np).astype(np.float32),
                "maskm": maskm_np,
            }
        )
    return in_maps


def _assemble(results):
    outf = np.zeros((TALL, D), dtype=np.float32)
    for c in range(NC):
        o = results[c]["out"]
        for g in range(NG):
            outf[2048 * g + 256 * c : 2048 * g + 256 * (c + 1)] = o[
                256 * g : 256 * (g + 1)
            ]
    return outf.reshape(B, T, D)


def kernel(x, Wq, Wk, Wv, Wproj, bproj, W1, b1, W2, b2):
    x = np.asarray(x, dtype=np.float32)
    if "nc" not in _CACHE:
        _CACHE["nc"] = build_nc(1)
    nc = _CACHE["nc"]
    in_maps = _prep_inputs(
        x, np.asarray(Wq), np.asarray(Wk), np.asarray(Wv), np.asarray(Wproj),
        np.asarray(bproj), np.asarray(W1), np.asarray(b1), np.asarray(W2),
        np.asarray(b2),
    )
    res = bass_utils.run_bass_kernel_spmd(nc, in_maps, list(range(NC)))
    return _assemble(res.results)


# revision 11
# speedup vs baseline: 4.6829x; 4.6829x over previous
"""Trainium2 Bass kernel for a dense transformer block (causal MHA + FFN).

Sharding: tensor-parallel over the 8 attention heads (1 head per core);
normalized per-head attention outputs (hs-major) are AllGathered (1MB/rank),
then every core computes the output projection + FFN for its own 1024
tokens (4 interleaved chunks of 256 so consumer work pipelines with the
causal attention sweep).

Shapes (hardcoded): B=2, T=4096, D=512, H=8, HS=64, FF=2048. 8 NeuronCores.
"""

import numpy as np
import ml_dtypes

import concourse.bass as bass
import concourse.mybir as mybir
from concourse import bacc
from concourse.tile import TileContext
from concourse import bass_utils
from concourse.masks import make_identity

B, T, D, H, HS = 2, 4096, 512, 8, 64
FF = 4 * D
NC = 8
TALL = B * T          # 8192 global tokens, batch-major
NTB = TALL // 512     # 16 token blocks of 512
NTT = TALL // 128     # 64 token tiles of 128
QB = T // 512         # 8 q-blocks of 512 per batch
KT = T // 128         # 32 k-tiles of 128 per batch
NG = 4                # AllGather groups (2048 tokens each)
OWN = TALL // NC      # 1024 tokens owned per core (4 chunks of 256)

BF16 = mybir.dt.bfloat16
F32 = mybir.dt.float32

_CACHE = {}


def build_nc(iters: int = 1, single_core: bool = False):
    nc = bacc.Bacc(
        "TRN2",
        target_bir_lowering=False,
        debug=False,
        num_devices=1 if single_core else NC,
    )

    xT = nc.dram_tensor("xT", [D, TALL], BF16, kind="ExternalInput")
    wqk = nc.dram_tensor("wqk", [128, 4 * 2 * HS], BF16, kind="ExternalInput")
    wv = nc.dram_tensor("wv", [128, 4 * HS], BF16, kind="ExternalInput")
    wprojF = nc.dram_tensor("wprojF", [128, 4 * D], BF16, kind="ExternalInput")
    w1 = nc.dram_tensor("w1", [128, 4 * FF], BF16, kind="ExternalInput")
    w2 = nc.dram_tensor("w2", [128, 16 * D], BF16, kind="ExternalInput")
    b1t = nc.dram_tensor("b1t", [128, 16], F32, kind="ExternalInput")
    b2t = nc.dram_tensor("b2t", [128, D], F32, kind="ExternalInput")
    xres = nc.dram_tensor("xres", [OWN, D], F32, kind="ExternalInput")
    maskm = nc.dram_tensor("maskm", [128, 4 * 512], BF16, kind="ExternalInput")
    out = nc.dram_tensor("out", [OWN, D], F32, kind="ExternalOutput")

    with TileContext(nc) as tc:
        with (
            tc.tile_pool(name="const", bufs=1) as constp,
            tc.tile_pool(name="qkv", bufs=1) as qkvp,
            tc.tile_pool(name="dram", bufs=1, space="DRAM") as dramp,
        ):
            # persistent constants
            wqk_s = constp.tile([128, 8 * HS], BF16, tag="wqk")
            wv_s = constp.tile([128, 4 * HS], BF16, tag="wv")
            wprojF_s = constp.tile([128, 4 * D], BF16, tag="wprojF")
            maskm_s = constp.tile([128, 4 * 512], BF16, tag="maskm")
            ident = constp.tile([128, 128], BF16, tag="ident")
            nc.sync.dma_start(out=wqk_s[:], in_=wqk[:])
            nc.sync.dma_start(out=wv_s[:], in_=wv[:])
            nc.sync.dma_start(out=wprojF_s[:], in_=wprojF[:])
            nc.sync.dma_start(out=maskm_s[:], in_=maskm[:])
            make_identity(nc, ident[:])

            # per-head attention tensors: qkT rows 0:64 = Q^T, 64:128 = K^T;
            # qkTs is the partition-swapped duplicate (rows 0:64 = K^T, 64:128 = Q^T)
            qkT = qkvp.tile([128, TALL], BF16, tag="qkT")
            qkTs = qkvp.tile([128, TALL], BF16, tag="qkTs")
            vP = qkvp.tile([128, NTT * (HS + 1)], BF16, tag="vP")

            for it in range(iters):
                ag_in = [dramp.tile([HS, 2048], BF16, tag=f"agin{g}_{it}", name=f"agin{g}_{it}")
                         for g in range(NG)]
                ag_out = [dramp.tile([H * HS, 2048], BF16, tag=f"agout{g}_{it}", name=f"agout{g}_{it}", addr_space="Shared")
                          for g in range(NG)]
                # ---------------- Phase 1: projections ----------------
                with (
                    tc.tile_pool(name="xt", bufs=1) as xtp,
                    tc.tile_pool(name="p1q", bufs=2, space="PSUM") as p1q,
                    tc.tile_pool(name="p1v", bufs=2, space="PSUM") as p1v,
                ):
                    xt_s = [xtp.tile([128, TALL], BF16, tag=f"xt{di}", name=f"xt{di}")
                            for di in range(4)]
                    for di in range(4):
                        nc.sync.dma_start(
                            out=xt_s[di][:], in_=xT[128 * di : 128 * (di + 1), :]
                        )
                    nc.vector.memset(vP[:, HS :: HS + 1], 1.0)

                    for tb in range(NTB):
                        sl = slice(512 * tb, 512 * (tb + 1))
                        pq = p1q.tile([128, 512], F32, tag="pq")
                        for di in range(4):
                            nc.tensor.matmul(
                                pq[:],
                                wqk_s[:, 2 * HS * di : 2 * HS * (di + 1)],
                                xt_s[di][:, sl],
                                start=(di == 0),
                                stop=(di == 3),
                            )
                        nc.vector.tensor_copy(qkT[:, sl], pq[:])
                        # swapped duplicate (Q<->K halves) for row-packed S^T
                        nc.sync.dma_start(out=qkTs[0:64, sl], in_=qkT[64:128, sl])
                        nc.sync.dma_start(out=qkTs[64:128, sl], in_=qkT[0:64, sl])
                        for tt4 in range(4):
                            tt = 4 * tb + tt4
                            pv = p1v.tile([128, HS], F32, tag="pv")
                            for di in range(4):
                                nc.tensor.matmul(
                                    pv[:],
                                    xt_s[di][:, 128 * tt : 128 * (tt + 1)],
                                    wv_s[:, HS * di : HS * (di + 1)],
                                    start=(di == 0),
                                    stop=(di == 3),
                                )
                            nc.vector.tensor_copy(
                                vP[:, (HS + 1) * tt : (HS + 1) * tt + HS], pv[:]
                            )

                # ---------------- Phases 2-4 ----------------
                with (
                    tc.tile_pool(name="sps", bufs=2, space="PSUM") as sps,
                    tc.tile_pool(name="avps", bufs=2, space="PSUM") as avps,
                    tc.tile_pool(name="smallps", bufs=2, space="PSUM") as smallps,
                    tc.tile_pool(name="work", bufs=4) as workp,
                    tc.tile_pool(name="work2", bufs=2) as work2p,
                    tc.tile_pool(name="big2", bufs=1) as big2p,
                ):
                    xres_s = big2p.tile([128, 8 * D], F32, tag="xres")
                    nc.sync.dma_start(
                        out=xres_s[:],
                        in_=xres.ap().rearrange("(a p) d -> p a d", p=128),
                    )
                    w1_s = big2p.tile([128, 4 * FF], BF16, tag="w1")
                    w2_s = big2p.tile([128, 16 * D], BF16, tag="w2")
                    b1t_s = big2p.tile([128, 16], F32, tag="b1t")
                    b2t_s = big2p.tile([128, D], F32, tag="b2t")
                    nc.sync.dma_start(out=w1_s[:], in_=w1[:])
                    nc.sync.dma_start(out=w2_s[:], in_=w2[:])
                    nc.sync.dma_start(out=b1t_s[:], in_=b1t[:])
                    nc.sync.dma_start(out=b2t_s[:], in_=b2t[:])
                    xmid = big2p.tile([128, 8 * D], F32, tag="xmid")
                    xmidT = big2p.tile([128, 4 * OWN], BF16, tag="xmidT")
                    h1T = big2p.tile([128, 16 * OWN], BF16, tag="h1T")

                    def attention(b, qb):
                        qsl = slice(T * b + 512 * qb, T * b + 512 * (qb + 1))
                        nkt = 4 * qb + 4
                        av = avps.tile([HS + 1, 512], F32, tag="av")
                        for pi in range(nkt // 2):
                            kt0, kt1 = 2 * pi, 2 * pi + 1
                            s = sps.tile([128, 1024], F32, tag="s")
                            nc.tensor.matmul(
                                s[:, 0:512],
                                qkTs[0:64, 128 * (KT * b + kt0) : 128 * (KT * b + kt0 + 1)],
                                qkT[0:64, qsl],
                                start=True,
                                stop=True,
                                tile_position=(0, 0),
                            )
                            nc.tensor.matmul(
                                s[:, 512:1024],
                                qkT[64:128, 128 * (KT * b + kt1) : 128 * (KT * b + kt1 + 1)],
                                qkTs[64:128, qsl],
                                start=True,
                                stop=True,
                                tile_position=(64, 0),
                            )
                            pT = workp.tile([128, 1024], BF16, tag="pT")
                            nc.scalar.activation(
                                pT[:], s[:], mybir.ActivationFunctionType.Exp
                            )
                            if kt1 >= 4 * qb:
                                j = kt0 - 4 * qb
                                nc.vector.tensor_mul(
                                    pT[:],
                                    pT[:],
                                    maskm_s[:, 512 * j : 512 * (j + 2)],
                                )
                            for half, kt in ((0, kt0), (1, kt1)):
                                gt = KT * b + kt
                                nc.tensor.matmul(
                                    av[:],
                                    vP[:, (HS + 1) * gt : (HS + 1) * (gt + 1)],
                                    pT[:, 512 * half : 512 * (half + 1)],
                                    start=(kt == 0),
                                    stop=(kt == nkt - 1),
                                )
                        # normalize at eviction: recip of denom row, broadcast,
                        # multiply into bf16 staging, ship to AllGather input
                        recip = work2p.tile([1, 512], F32, tag="recip")
                        nc.vector.reciprocal(recip[:], av[HS : HS + 1, :])
                        bc = work2p.tile([64, 512], F32, tag="bc")
                        nc.gpsimd.partition_broadcast(bc[:], recip[:])
                        ag_stage = workp.tile([64, 512], BF16, tag="ag_stage")
                        nc.vector.tensor_mul(ag_stage[:], av[0:HS, :], bc[:])
                        g = 2 * b + qb // 4
                        col = (T * b + 512 * qb) % 2048
                        nc.sync.dma_start(
                            out=ag_in[g][:, col : col + 512], in_=ag_stage[:]
                        )

                    def ag_fire(g):
                        if single_core:
                            nc.sync.dma_start(
                                out=ag_out[g][0:HS, :], in_=ag_in[g][:]
                            )
                            return
                        nc.gpsimd.collective_compute(
                            "AllGather",
                            mybir.AluOpType.bypass,
                            replica_groups=[list(range(NC))],
                            ins=[ag_in[g].opt()],
                            outs=[ag_out[g].opt()],
                        )

                    def consume(g):
                        # owned tokens of this group: cols [256*rank, 256*(rank+1))
                        agT = work2p.tile([128, 4 * 256], BF16, tag="agT")
                        col0 = nc.sync.partition_id() * 256
                        nc.sync.dma_start(
                            out=agT[:],
                            in_=ag_out[g][:, bass.ds(col0, 256)].rearrange(
                                "(r p) j -> p r j", p=128
                            ),
                        )
                        for tt in range(2):
                            ti = 2 * g + tt
                            pp = smallps.tile([128, D], F32, tag="small")
                            for r in range(4):
                                nc.tensor.matmul(
                                    pp[:],
                                    agT[:, 256 * r + 128 * tt : 256 * r + 128 * (tt + 1)],
                                    wprojF_s[:, 512 * r : 512 * (r + 1)],
                                    start=(r == 0),
                                    stop=(r == 3),
                                )
                            csl = slice(D * ti, D * (ti + 1))
                            nc.vector.tensor_add(xmid[:, csl], pp[:], xres_s[:, csl])
                            xb = workp.tile([128, D], BF16, tag="xb")
                            nc.vector.tensor_copy(xb[:], xmid[:, csl])
                            for di in range(4):
                                tr = smallps.tile([128, 128], BF16, tag="small")
                                nc.tensor.transpose(
                                    tr[:], xb[:, 128 * di : 128 * (di + 1)], ident[:]
                                )
                                nc.vector.tensor_copy(
                                    xmidT[:, OWN * di + 128 * ti : OWN * di + 128 * (ti + 1)],
                                    tr[:],
                                )
                        # FFN over this group's 256 tokens
                        for hi in range(16):
                            f1 = smallps.tile([128, 256], F32, tag="small")
                            for di in range(4):
                                nc.tensor.matmul(
                                    f1[:],
                                    w1_s[:, FF * di + 128 * hi : FF * di + 128 * (hi + 1)],
                                    xmidT[:, OWN * di + 256 * g : OWN * di + 256 * (g + 1)],
                                    start=(di == 0),
                                    stop=(di == 3),
                                )
                            nc.vector.tensor_scalar(
                                h1T[:, OWN * hi + 256 * g : OWN * hi + 256 * (g + 1)],
                                f1[:],
                                b1t_s[:, hi : hi + 1],
                                0.0,
                                op0=mybir.AluOpType.add,
                                op1=mybir.AluOpType.max,
                            )
                        for tt in range(2):
                            ti = 2 * g + tt
                            f2 = smallps.tile([128, D], F32, tag="small")
                            for hi in range(16):
                                nc.tensor.matmul(
                                    f2[:],
                                    h1T[:, OWN * hi + 128 * ti : OWN * hi + 128 * (ti + 1)],
                                    w2_s[:, D * hi : D * (hi + 1)],
                                    start=(hi == 0),
                                    stop=(hi == 15),
                                )
                            osum = workp.tile([128, D], F32, tag="osum")
                            nc.vector.tensor_add(
                                osum[:], f2[:], xmid[:, D * ti : D * (ti + 1)]
                            )
                            nc.vector.tensor_add(osum[:], osum[:], b2t_s[:])
                            nc.sync.dma_start(
                                out=out[128 * ti : 128 * (ti + 1), :], in_=osum[:]
                            )

                    order = []
                    for b in range(B):
                        for qb in range(QB):
                            order.append((b, qb))
                    for i, (b, qb) in enumerate(order):
                        attention(b, qb)
                        if qb % 4 == 3:
                            g = 2 * b + qb // 4
                            ag_fire(g)
                            if g >= 1:
                                consume(g - 1)
                    consume(NG - 1)

    nc.compile()
    return nc


def _prep_inputs(x, Wq, Wk, Wv, Wproj, bproj, W1, b1, W2, b2):
    bf16 = ml_dtypes.bfloat16
    xf = np.ascontiguousarray(x.reshape(TALL, D).astype(np.float32))
    xT_np = np.ascontiguousarray(xf.T).astype(bf16)
    w1_np = np.ascontiguousarray(
        W1.reshape(4, 128, FF).transpose(1, 0, 2).reshape(128, 4 * FF)
    ).astype(bf16)
    w2_np = np.ascontiguousarray(
        W2.reshape(16, 128, D).transpose(1, 0, 2).reshape(128, 16 * D)
    ).astype(bf16)
    wprojF_np = np.ascontiguousarray(
        Wproj.reshape(4, 128, D).transpose(1, 0, 2).reshape(128, 4 * D)
    ).astype(bf16)
    b1t_np = np.ascontiguousarray(b1.reshape(16, 128).T).astype(np.float32)
    b2t_np = np.ascontiguousarray(np.broadcast_to(b2, (128, D))).astype(np.float32)

    ki = np.arange(128)[:, None]
    qi = np.arange(512)[None, :]
    maskm_np = np.concatenate(
        [(qi >= 128 * j + ki) for j in range(4)], axis=1
    ).astype(bf16)

    scale = HS ** -0.5
    in_maps = []
    for c in range(NC):
        wq_c = (Wq[c] * scale).reshape(4, 128, HS)
        wk_c = Wk[c].reshape(4, 128, HS)
        wqk_np = np.ascontiguousarray(
            np.concatenate([wq_c, wk_c], axis=2).transpose(1, 0, 2).reshape(128, 8 * HS)
        ).astype(bf16)
        wv_np = np.ascontiguousarray(
            Wv[c].reshape(4, 128, HS).transpose(1, 0, 2).reshape(128, 4 * HS)
        ).astype(bf16)
        xres_np = np.concatenate(
            [xf[2048 * g + 256 * c : 2048 * g + 256 * (c + 1)] for g in range(NG)]
        ) + bproj.astype(np.float32)
        in_maps.append(
            {
                "xT": xT_np,
                "wqk": wqk_np,
                "wv": wv_np,
                "wprojF": wprojF_np,
                "w1": w1_np,
                "w2": w2_np,
                "b1t": b1t_np,
                "b2t": b2t_np,
                "xres": np.ascontiguousarray(xres_np).astype(np.float32),
                "maskm": maskm_np,
            }
        )
    return in_maps


def _assemble(results):
    outf = np.zeros((TALL, D), dtype=np.float32)
    for c in range(NC):
        o = results[c]["out"]
        for g in range(NG):
            outf[2048 * g + 256 * c : 2048 * g + 256 * (c + 1)] = o[
                256 * g : 256 * (g + 1)
            ]
    return outf.reshape(B, T, D)


def kernel(x, Wq, Wk, Wv, Wproj, bproj, W1, b1, W2, b2):
    x = np.asarray(x, dtype=np.float32)
    if "nc" not in _CACHE:
        _CACHE["nc"] = build_nc(1)
    nc = _CACHE["nc"]
    in_maps = _prep_inputs(
        x, np.asarray(Wq), np.asarray(Wk), np.asarray(Wv), np.asarray(Wproj),
        np.asarray(bproj), np.asarray(W1), np.asarray(b1), np.asarray(W2),
        np.asarray(b2),
    )
    res = bass_utils.run_bass_kernel_spmd(nc, in_maps, list(range(NC)))
    return _assemble(res.results)
